# revision 4
# baseline (speedup 1.0000x reference)
"""Cross-attention kernel for TRN2, 8 NeuronCores, data-parallel over points.

Math (derived from the reference):
  qp[n]  = Wq @ q[n]                      (needed for the residual)
  scores[n,w] = (Wq q[n]) . (Wk k[w,n]) * s = (q[n] @ (Wq.T Wk) * s) . k[w,n]
  attn[n] = softmax_w(scores[n])          (identical for every query view)
  vmix[n] = sum_w attn[n,w] * v[w,n]      (mix RAW v, then project once)
  y[n]    = gelu(vmix[n] @ (Wo Wv).T + bo) + qp[n]
  out[c][8*i + j] = y[c*4096 + i]         (row replicated 8x, per view slab c)

Per core: 4096 points, 32 tiles of 128 partition-points.
"""

import numpy as np

import concourse.bass as bass
import concourse.mybir as mybir
import concourse.tile as tile
from concourse import bacc
from concourse.bass_utils import run_bass_kernel_spmd

N_CORES = 8
N_TOTAL = 32768
NC_PTS = N_TOTAL // N_CORES  # 4096 points per core
D = 256
V = 8
P = 128
N_TILES = NC_PTS // P  # 32
F32 = mybir.dt.float32
AX = mybir.AxisListType
OP = mybir.AluOpType
AF = mybir.ActivationFunctionType


def build_bass(n_tiles: int = N_TILES, gelu: bool = True):
    nc = bacc.Bacc(
        "TRN2", target_bir_lowering=False, debug=False, num_devices=N_CORES
    )
    npts = n_tiles * P
    q_d = nc.dram_tensor("q", [npts, D], F32, kind="ExternalInput")
    k_d = nc.dram_tensor("k", [V, npts, D], F32, kind="ExternalInput")
    v_d = nc.dram_tensor("v", [V, npts, D], F32, kind="ExternalInput")
    ma_d = nc.dram_tensor("ma", [D, D], F32, kind="ExternalInput")
    mq_d = nc.dram_tensor("mq", [D, D], F32, kind="ExternalInput")
    mb_d = nc.dram_tensor("mb", [D, D], F32, kind="ExternalInput")
    bo_d = nc.dram_tensor("bo_b", [P, D], F32, kind="ExternalInput")
    id_d = nc.dram_tensor("ident", [P, P], F32, kind="ExternalInput")
    out_d = nc.dram_tensor("out", [npts * V, D], F32, kind="ExternalOutput")

    with tile.TileContext(nc) as tc:
        with (
            tc.tile_pool(name="singles", bufs=1) as singles,
            tc.tile_pool(name="io", bufs=3) as io,
            tc.tile_pool(name="work", bufs=2) as work,
            tc.tile_pool(name="pst", bufs=1, space="PSUM") as pst,
            tc.tile_pool(name="ps", bufs=2, space="PSUM") as ps,
        ):
            # Weights as [din_half, dout] halves for K=256 accumulation.
            ma_t = singles.tile([P, 2, D], F32)
            mq_t = singles.tile([P, 2, D], F32)
            mb_t = singles.tile([P, 2, D], F32)
            bo_t = singles.tile([P, D], F32)
            id_t = singles.tile([P, P], F32)
            for t, d in ((ma_t, ma_d), (mq_t, mq_d), (mb_t, mb_d)):
                nc.sync.dma_start(
                    out=t, in_=d.ap().rearrange("(h p) d -> p h d", p=P)
                )
            nc.sync.dma_start(out=bo_t, in_=bo_d.ap())
            nc.sync.dma_start(out=id_t, in_=id_d.ap())

            for i in range(n_tiles):
                sl = slice(i * P, (i + 1) * P)
                q_t = io.tile([P, D], F32, tag="q")
                k_t = io.tile([P, V, D], F32, tag="k")
                v_t = io.tile([P, V, D], F32, tag="v")
                nc.sync.dma_start(out=q_t, in_=q_d.ap()[sl])
                nc.sync.dma_start(
                    out=k_t, in_=k_d.ap()[:, sl].rearrange("w p d -> p w d")
                )
                nc.sync.dma_start(
                    out=v_t, in_=v_d.ap()[:, sl].rearrange("w p d -> p w d")
                )

                # q tile -> [din, n] halves via PE transpose (stationary operand)
                qT_ps = pst.tile([P, 2, P], F32, tag="qT_ps")
                nc.tensor.transpose(qT_ps[:, 0], q_t[:, 0:P], id_t)
                nc.tensor.transpose(qT_ps[:, 1], q_t[:, P:D], id_t)
                qT_t = work.tile([P, 2, P], F32, tag="qT")
                nc.vector.tensor_copy(qT_t, qT_ps)

                # qk = q @ (Wq.T Wk * s)  [point-major], qp = q @ Wq.T
                qk_ps = ps.tile([P, D], F32, tag="qk")
                qp_ps = ps.tile([P, D], F32, tag="qp")
                nc.tensor.matmul(qk_ps, qT_t[:, 0], ma_t[:, 0], start=True, stop=False)
                nc.tensor.matmul(qk_ps, qT_t[:, 1], ma_t[:, 1], start=False, stop=True)
                nc.tensor.matmul(qp_ps, qT_t[:, 0], mq_t[:, 0], start=True, stop=False)
                nc.tensor.matmul(qp_ps, qT_t[:, 1], mq_t[:, 1], start=False, stop=True)

                # scores[n,w] = sum_d qk[n,d] * k[w,n,d]
                # (tensor_tensor_reduce is broken on this HW path -> mul+reduce)
                scores_t = work.tile([P, V], F32, tag="scores")
                scr = work.tile([P, V, D], F32, tag="scr")
                for w in range(V):
                    nc.vector.tensor_mul(scr[:, w], qk_ps, k_t[:, w])
                nc.vector.tensor_reduce(
                    scores_t, scr, axis=AX.X, op=OP.add
                )

                # softmax over the 8 views (per partition-point)
                mx = work.tile([P, 1], F32, tag="mx")
                nc.vector.tensor_reduce(mx, scores_t, axis=AX.X, op=OP.max)
                nc.vector.tensor_scalar(
                    out=scores_t, in0=scores_t, scalar1=mx, scalar2=None,
                    op0=OP.subtract,
                )
                attn_t = work.tile([P, V], F32, tag="attn")
                sm = work.tile([P, 1], F32, tag="sm")
                nc.scalar.activation(attn_t, scores_t, AF.Exp, accum_out=sm)
                rs = work.tile([P, 1], F32, tag="rs")
                nc.vector.reciprocal(rs, sm)
                nc.vector.tensor_scalar(
                    out=attn_t, in0=attn_t, scalar1=rs, scalar2=None, op0=OP.mult
                )

                # vmix = sum_w attn[:,w] * v[:,w,:]  (ACT scales, DVE accumulates)
                vmix_t = work.tile([P, D], F32, tag="vmix")
                nc.scalar.activation(
                    vmix_t, v_t[:, 0], AF.Copy, scale=attn_t[:, 0:1]
                )
                for w in range(1, V):
                    vw = work.tile([P, D], F32, tag="vw", bufs=3)
                    nc.scalar.activation(
                        vw, v_t[:, w], AF.Copy, scale=attn_t[:, w : w + 1]
                    )
                    nc.vector.tensor_add(vmix_t, vmix_t, vw)

                # vmix -> [din, n] halves for the output projection
                vT_ps = pst.tile([P, 2, P], F32, tag="vT_ps")
                nc.tensor.transpose(vT_ps[:, 0], vmix_t[:, 0:P], id_t)
                nc.tensor.transpose(vT_ps[:, 1], vmix_t[:, P:D], id_t)
                vT_t = work.tile([P, 2, P], F32, tag="vT")
                nc.vector.tensor_copy(vT_t, vT_ps)

                # ylin = vmix @ (Wo Wv).T
                y_ps = ps.tile([P, D], F32, tag="y")
                nc.tensor.matmul(y_ps, vT_t[:, 0], mb_t[:, 0], start=True, stop=False)
                nc.tensor.matmul(y_ps, vT_t[:, 1], mb_t[:, 1], start=False, stop=True)

                # y = gelu(ylin + bo) + qp
                t1 = work.tile([P, D], F32, tag="t1")
                nc.vector.tensor_add(t1, y_ps, bo_t)
                g = work.tile([P, D], F32, tag="g")
                nc.scalar.activation(g, t1, AF.Gelu if gelu else AF.Identity)
                y_out = io.tile([P, D], F32, tag="yout")
                nc.vector.tensor_add(y_out, g, qp_ps)

                # store: each point row replicated 8x -> 8KB contiguous/partition
                dst = out_d.ap()[i * P * V : (i + 1) * P * V].rearrange(
                    "(p r) d -> p r d", r=V
                )
                src = bass.AP(
                    tensor=y_out.tensor,
                    offset=y_out.offset,
                    ap=[y_out.ap[0], [0, V], *y_out.ap[1:]],
                )
                nc.sync.dma_start(out=dst, in_=src)

    nc.compile()
    return nc


_NC_CACHE = {}


def _get_nc(n_tiles: int = N_TILES):
    if n_tiles not in _NC_CACHE:
        _NC_CACHE[n_tiles] = build_bass(n_tiles)
    return _NC_CACHE[n_tiles]


def _host_prep(Wq, Wk, Wv, Wo, bo):
    Wq = np.asarray(Wq, dtype=np.float32)
    Wk = np.asarray(Wk, dtype=np.float32)
    Wv = np.asarray(Wv, dtype=np.float32)
    Wo = np.asarray(Wo, dtype=np.float32)
    bo = np.asarray(bo, dtype=np.float32)
    scale = np.float32(1.0) / np.sqrt(np.float32(D))
    ma = np.ascontiguousarray((Wq.T @ Wk) * scale, dtype=np.float32)
    mq = np.ascontiguousarray(Wq.T, dtype=np.float32)
    mb = np.ascontiguousarray(Wv.T @ Wo.T, dtype=np.float32)
    bo_b = np.ascontiguousarray(np.broadcast_to(bo, (P, D)), dtype=np.float32)
    ident = np.eye(P, dtype=np.float32)
    return ma, mq, mb, bo_b, ident


def make_in_maps(q, k, v, Wq, Wk, Wv, Wo, bo):
    q = np.asarray(q, dtype=np.float32)
    k = np.asarray(k, dtype=np.float32)
    v = np.asarray(v, dtype=np.float32)
    ma, mq, mb, bo_b, ident = _host_prep(Wq, Wk, Wv, Wo, bo)
    in_maps = []
    for c in range(N_CORES):
        sl = slice(c * NC_PTS, (c + 1) * NC_PTS)
        in_maps.append(
            {
                "q": np.ascontiguousarray(q[0, sl]),
                "k": np.ascontiguousarray(k[:, sl]),
                "v": np.ascontiguousarray(v[:, sl]),
                "ma": ma,
                "mq": mq,
                "mb": mb,
                "bo_b": bo_b,
                "ident": ident,
            }
        )
    return in_maps


def kernel(q, k, v, Wq, Wk, Wv, Wo, bo):
    nc = _get_nc()
    in_maps = make_in_maps(q, k, v, Wq, Wk, Wv, Wo, bo)
    res = run_bass_kernel_spmd(nc, in_maps, core_ids=list(range(N_CORES)))
    return np.stack([r["out"] for r in res.results], axis=0)
